# revision 84
# baseline (speedup 1.0000x reference)
"""GCN (3-layer, PyG GCNConv semantics) on 8 Trainium2 NeuronCores.

Strategy (graph/data parallel, dst-sharded), v3:
  - Nodes sharded across 8 cores (rows of x / output).
  - Per layer: each core computes its slice of h = y_prev @ W on PE,
    scales rows by dis[node] (1/sqrt(deg), folded into the PSUM->SBUF
    copy as a per-partition tensor_scalar mult), writes it (fp16,
    256B-strided rows) to a per-layer shard buffer, then sub-AllGathers
    into a per-layer SHARED-scratchpad gfull table (write-once across
    the 8 cores instead of 8 per-core copies).
  - Aggregation: edges bucketed by (dst tile, src class); chunks of 128
    edges; dma_gather (custom 128B-payload lowering) fetches g[src]
    rows in large calls (up to NBMAX blocks per call; the SWDGE
    descriptor ring is enlarged via dynamic_dma_scratch_size to allow
    ~1792 rows/call, amortizing the ~1us fixed descriptor-gen cost).
  - The scatter-add becomes PE matmuls agg_T += G.T @ S where S is a
    HOST-PRECOMPUTED one-hot whose values are dis[dst] (so that
    message = dis[src]*h[src] matmul S gives the symmetric norm);
    S lives in DRAM and is DMA'd per tile per layer (no vector build).
    Self-loops are a diag chunk whose lhsT is the local (dis-scaled)
    activation tile and whose S diag is also dis[dst].
  - Epilogue: relu(agg + b) in one ScalarE activation (transposed
    layout: bias is per-partition). Final layer: log_softmax via exp
    (ACT), partition-sum (PE ones-matmul), ln (ACT), broadcast (PE),
    subtract.
  - Output is produced transposed [40, nodes] per core; host transposes.

Self-contained: only needs numpy + the concourse stack at /opt/trn_rl_repo.
"""

import os
import sys

sys.path.insert(0, "/opt/trn_rl_repo")

import numpy as np

import concourse.bacc as bacc
import concourse.tile as tile
import concourse.mybir as mybir
from concourse import ap_utils
from concourse.bass import AP, MemorySpace
from concourse.bass_utils import run_bass_kernel_spmd

fp32 = mybir.dt.float32
fp16 = mybir.dt.float16
i16 = mybir.dt.int16

N_CORES = 8
P = 128
NBMAX = int(os.environ.get("GCN_NBMAX", "8"))
DMA_SCRATCH = int(os.environ.get("GCN_SCRATCH", "16384"))
SINGLE_PKT = os.environ.get("GCN_SP", "1") == "1"
SL_CAP = 4095      # max slice rows: N_CORES*SL must fit int16 gather index
NQ = 4             # SWDGE queues

# test.py sets this to capture profile info; harness leaves it off.
TRACE = os.environ.get("GCN_TRACE", "0") == "1"
USE_SHARED = os.environ.get("GCN_SHARED", "1") == "1"
LAST_RESULT = None


# ---------------------------------------------------------------- gather ----
def dma_gather_raw(engine, out_ap, in_ap, idxs_ap, num_idxs, elem_size, elem_step,
                   queue_num=0, single_packet=True):
    """bass dma_gather minus the elem_size%256B assert: the ucode only needs
    the row STRIDE 256B-quantized; the payload is free."""
    assert idxs_ap.dtype == mybir.dt.int16
    assert in_ap.space == MemorySpace.DRAM
    assert out_ap.space == MemorySpace.SBUF
    assert in_ap.dtype == out_ap.dtype
    assert ap_utils.ap_is_contiguous(out_ap.ap[1:])
    assert ap_utils.ap_is_contiguous(idxs_ap.ap[1:])
    assert in_ap.ap[-1][1] == elem_size
    assert out_ap.ap[-1][1] == elem_size
    assert in_ap.ap[0][0] == elem_step
    stride_bytes = elem_step * mybir.dt.size(in_ap.dtype)
    assert stride_bytes % 256 == 0
    return engine.add_instruction(
        mybir.InstDMAGatherAnt(
            name=engine.bass.get_next_instruction_name(),
            ins=[
                *engine.lower_ap_dma(in_ap, for_custom_bir_dma=True),
                engine.lower_ap(idxs_ap),
                engine.lower_val_access(engine.to_reg(num_idxs)),
            ],
            outs=[engine.lower_ap(out_ap)],
            transpose=False,
            num_idxs=num_idxs,
            elem_size=elem_size,
            stride_bytes_256=stride_bytes // 256,
            gen_mode=0,
            single_packet=single_packet,
            queue_num=queue_num,
            sbuf_tokens_per_rank=0,
            sbuf_free_dim_per_rank=0,
            sbuf_free_dim_pad_per_rank=0,
            sbuf_byte_offset=0,
        )
    )


# ---------------------------------------------------------- host preprocess --
def _prepare_spmd(edge_index, n, npc):
    """Shared (max-over-cores) chunk layout so all cores run one program.

    Src classes: 0 = local (own shard, gathered from the per-layer shard
    buffer pre-collective), 1+k = slice k of gfull.  The AllGather is split
    into NR sub-collectives; sub-AG k gathers every core's local rows
    [k*SL, (k+1)*SL) into a contiguous gfull region of N_CORES*SL rows
    ordered (core, row%SL), so slice-k gathers can start as soon as
    sub-AG k lands (int16 gather indices are region-relative).

    Also host-builds the per-core S tables: for each chunk, S[slot, d] =
    dis[dst] at the edge's dst-local column (one-hot rows), plus a diag
    chunk per tile with S[d, d] = dis[dst].  Messages are dis[src]-scaled
    on-chip, so G.T @ S accumulates dis[src]*dis[dst]*h[src] per dst.
    """
    src = np.asarray(edge_index[0], np.int64)
    dst = np.asarray(edge_index[1], np.int64)
    deg = np.bincount(dst, minlength=n).astype(np.float64) + 1.0
    dis64 = 1.0 / np.sqrt(deg)
    dis = dis64.astype(np.float32)

    # dedup repeated (s, d) edges (randint graphs have ~6%): a repeated edge
    # contributes m * dis[s] * dis[d] * h[s]; fold m into the S value
    eid = src * n + dst
    uniq, ucnt = np.unique(eid, return_counts=True)
    src = uniq // n
    dst = uniq % n
    emult = ucnt.astype(np.float32)

    NR = (npc + SL_CAP - 1) // SL_CAP
    while npc % NR != 0 and NR < npc:
        NR += 1
    SL = npc // NR
    n_tiles = (npc + P - 1) // P
    NCLS = 1 + NR

    # per-core bucket counts
    cnts = np.zeros((N_CORES, n_tiles, NCLS), np.int64)
    core_edges = []
    for c in range(N_CORES):
        m = (dst >= c * npc) & (dst < (c + 1) * npc)
        s_c, d_c, w_c = src[m], dst[m] - c * npc, emult[m]
        tile_c = d_c // P
        own = (s_c >= c * npc) & (s_c < (c + 1) * npc)
        src_core = s_c // npc
        src_loc = s_c - src_core * npc
        slc = src_loc // SL
        cls = np.where(own, 0, 1 + slc)
        rel = np.where(own, src_loc,
                       src_core * SL + (src_loc - slc * SL))
        order = np.lexsort((s_c, cls, tile_c))
        tile_s, cls_s = tile_c[order], cls[order]
        bucket = tile_s * NCLS + cls_s
        cnts[c] = np.bincount(bucket, minlength=n_tiles * NCLS).reshape(
            n_tiles, NCLS)
        core_edges.append((bucket, rel[order], (d_c[order] - tile_s * P),
                           w_c[order]))

    # --- zero-ceil piece packing ---------------------------------------
    # Per class, runs (t, q) of cnt_max = max-over-core edges are laid out
    # back-to-back in one slot stream; 128-slot blocks are the gather /
    # matmul-lhsT unit, and a run crossing a block boundary is split into
    # PIECES (variable-K matmuls).  No per-(tile,class) ceil padding: only
    # the SPMD max-over-cores spread and per-class call tails pad.
    cnt_max = cnts.max(axis=0)  # [n_tiles, NCLS]
    block_base = np.zeros(NCLS + 1, np.int64)   # global block id per class
    run_start = np.zeros((n_tiles, NCLS), np.int64)  # class-local slot start
    g = 0
    for q in range(NCLS):
        block_base[q] = g
        pos = 0
        for t in range(n_tiles):
            run_start[t, q] = pos
            pos += int(cnt_max[t, q])
        g += (pos + P - 1) // P
    block_base[NCLS] = g
    nchunk_g = g

    # pieces per tile: (block, row_off, K, q), in class order then slot order
    pieces_of_tile = [[] for _ in range(n_tiles)]
    run_piece_base = np.zeros((n_tiles, NCLS), np.int64)
    block_tile_of = np.full(max(nchunk_g, 1), -1, np.int64)
    for q in range(NCLS):
        for t in range(n_tiles):
            s0 = int(run_start[t, q])
            cntq = int(cnt_max[t, q])
            run_piece_base[t, q] = len(pieces_of_tile[t])
            while cntq > 0:
                ro = s0 % P
                K = min(P - ro, cntq)
                blk = int(block_base[q] + s0 // P)
                if block_tile_of[blk] < 0:
                    block_tile_of[blk] = t  # first tile served by this block
                pieces_of_tile[t].append((blk, int(ro), int(K), q))
                s0 += K
                cntq -= K
    block_tile_of[block_tile_of < 0] = 0

    # calls: per class, maximally-full NBMAX-block runs; local class first,
    # then slice calls sorted by (first tile served, class) so gpsimd's
    # in-order stream matches tile-major consumption
    def class_calls(q):
        out = []
        lo = int(block_base[q])
        hi = int(block_base[q + 1])
        while lo < hi:
            nb = min(NBMAX, hi - lo)
            out.append((q, lo, nb, False))
            lo += nb
        return out

    calls = class_calls(0)
    n_local_calls = len(calls)
    slice_calls = []
    for q in range(1, NCLS):
        slice_calls.extend(class_calls(q))
    slice_calls.sort(key=lambda c: (int(block_tile_of[c[1]]), c[0]))
    assert all(not c[3] for c in calls + slice_calls)
    calls = calls + slice_calls
    ranges = []

    # S layout: per tile, one 128-col block per piece, then the diag block
    tile_off = np.zeros(n_tiles + 1, np.int64)
    for t in range(n_tiles):
        tile_off[t + 1] = tile_off[t] + len(pieces_of_tile[t]) + 1
    nchunk_all = int(tile_off[n_tiles])

    per_core = []
    for c in range(N_CORES):
        bucket, rel_s, dl_s, w_s = core_edges[c]
        cnt = cnts[c].reshape(-1)
        bs = np.concatenate([[0], np.cumsum(cnt)[:-1]])
        rank = np.arange(len(bucket)) - bs[bucket]
        t_of = bucket // NCLS
        q_of = bucket % NCLS
        # class-local slot of this edge, then global gather slot via blocks
        cslot = run_start[t_of, q_of] + rank
        gslot = block_base[q_of] * P + cslot
        slot = gslot % P

        idx_flat = np.zeros(max(nchunk_g, 1) * P, np.int64)
        idx_flat[gslot] = rel_s

        # host-built S table [P, nchunk_all*P] fp16: edge's piece within its
        # run = how many block boundaries its class-slot has crossed
        S = np.zeros((P, nchunk_all * P), np.float16)
        piece_idx = run_piece_base[t_of, q_of] + \
            (cslot // P - run_start[t_of, q_of] // P)
        ecol = (tile_off[t_of] + piece_idx) * P + dl_s
        vals = (w_s * dis[c * npc + t_of * P + dl_s]).astype(np.float16)
        S[slot, ecol] = vals
        # diag blocks
        for t in range(n_tiles):
            tw = min(P, npc - t * P)
            dcol = (tile_off[t] + len(pieces_of_tile[t])) * P
            dd = np.arange(tw)
            S[dd, dcol + dd] = dis[c * npc + t * P + dd].astype(np.float16)

        # dis column table [P, n_tiles] fp32 (per-partition scale in h_tile)
        dis_col = np.zeros((P, n_tiles), np.float32)
        node = c * npc + np.arange(npc)
        dis_col[np.arange(npc) % P, np.arange(npc) // P] = dis[node]

        tmp = idx_flat.astype(np.int16).reshape(max(nchunk_g, 1) * 8, 16).T
        idx16 = np.tile(np.ascontiguousarray(tmp), (8, 1))
        per_core.append(dict(idx16=idx16, S=S, dis_col=dis_col))

    struct = dict(n_tiles=n_tiles, NR=NR, SL=SL, ranges=ranges, calls=calls,
                  nchunk_g=nchunk_g, nchunk_all=nchunk_all,
                  pieces_of_tile=pieces_of_tile, tile_off=tile_off,
                  n_local_calls=n_local_calls,
                  max_nb=max(c[2] for c in calls) if calls else 1)
    return struct, per_core


# ----------------------------------------------------------------- program --
def _build(struct, n, npc, f_in, f_hid, f_out):
    nt = struct["n_tiles"]
    NR = struct["NR"]
    SL = struct["SL"]
    nchunk_g = struct["nchunk_g"]
    nchunk_all = struct["nchunk_all"]
    tile_off = struct["tile_off"]
    maxb = struct["max_nb"]
    maxcpt = max(len(c) for c in struct["pieces_of_tile"]) + 1
    fdims = [(f_in, f_hid), (f_hid, f_hid), (f_hid, f_out)]
    ic = max(nchunk_g, 1) * 8

    nc = bacc.Bacc("TRN2", target_bir_lowering=False, debug=False,
                   num_devices=N_CORES, num_swdge_queues=NQ,
                   dynamic_dma_scratch_size=DMA_SCRATCH)
    xT = nc.dram_tensor("xT", [f_in, npc], fp16, kind="ExternalInput").ap()
    Ws = [nc.dram_tensor(f"W{i+1}", [fi, fo], fp16, kind="ExternalInput").ap()
          for i, (fi, fo) in enumerate(fdims)]
    bs = [nc.dram_tensor(f"b{i+1}", [fo, 1], fp32, kind="ExternalInput").ap()
          for i, (_, fo) in enumerate(fdims)]
    idx_in = nc.dram_tensor("idx_all", [P, ic], i16, kind="ExternalInput").ap()
    S_in = nc.dram_tensor("S_all", [P, nchunk_all * P], fp16,
                          kind="ExternalInput").ap()
    dis_in = nc.dram_tensor("dis_col", [P, nt], fp32, kind="ExternalInput").ap()
    ones_in = nc.dram_tensor("ones40", [f_out, 1], fp32, kind="ExternalInput").ap()
    ones16_in = nc.dram_tensor("ones40h", [f_out, 1], fp16, kind="ExternalInput").ap()
    out3T = nc.dram_tensor("out3T", [f_out, npc], fp32, kind="ExternalOutput").ap()

    with tile.TileContext(nc) as tc:
        with (
            tc.tile_pool(name="const", bufs=1) as cp,
            tc.tile_pool(name="gather", bufs=struct["n_local_calls"] + 20) as gp,
            tc.tile_pool(name="sel", bufs=8) as selp,
            tc.tile_pool(name="work", bufs=3) as wp,
            tc.tile_pool(name="persist", bufs=1) as pp,
            tc.tile_pool(name="psA", bufs=3, space="PSUM") as psA,
            tc.tile_pool(name="psB", bufs=1, space="PSUM") as psB,
            tc.tile_pool(name="psC", bufs=2, space="PSUM") as psC,
            tc.tile_pool(name="dram", bufs=1, space="DRAM") as dr,
        ):
            # idx table split-loaded so the first gather calls only wait for
            # their own slice of the 2.6MB table, not the whole DMA
            idx_sb = cp.tile([P, ic], i16)
            ISPL = max(1, ic // 8)
            for o in range(0, ic, ISPL):
                w = min(ISPL, ic - o)
                nc.sync.dma_start(idx_sb[:, o: o + w], idx_in[:, o: o + w])
            dis_sb = cp.tile([P, nt], fp32)
            nc.sync.dma_start(dis_sb[:], dis_in[:])
            W_sb = []
            b_sb = []
            for i, (fi, fo) in enumerate(fdims):
                wt = cp.tile([fi, fo], fp16, tag=f"W{i}")
                nc.sync.dma_start(wt[:], Ws[i][:])
                W_sb.append(wt[:])
                b = cp.tile([fo, 1], fp32, tag=f"b{i}")
                nc.sync.dma_start(b[:], bs[i][:])
                b_sb.append(b)
            ones_col = cp.tile([f_out, 1], fp16)
            nc.sync.dma_start(ones_col[:], ones16_in[:])
            ones_row = cp.tile([1, f_out], fp32)
            nc.sync.dma_start(ones_row[:], ones_in[:].transpose([1, 0]))

            # split the xT load so early h tiles don't wait for the full 3.2MB
            xT_sb = pp.tile([f_in, npc], fp16, tag="xT")
            XSPLIT = max(1, nt // 8)
            for o in range(0, npc, XSPLIT * P):
                w = min(XSPLIT * P, npc - o)
                nc.sync.dma_start(xT_sb[:, o: o + w], xT[:, o: o + w])
            yT0 = pp.tile([f_hid, nt * P], fp16, tag="yT0")
            yT1 = pp.tile([f_hid, nt * P], fp16, tag="yT1")
            yT = [yT0[:], yT1[:]]

            # per-layer shard + shared gfull (no cross-layer WAR at all)
            shard_d = [dr.tile([npc, 128], fp16, tag=f"shard{l}",
                               name=f"shard{l}_d")
                       for l in range(3)]
            # one shared tensor per (layer, slice region): the tile framework
            # allows only a single writing instruction per Shared DRAM tensor
            RW = N_CORES * SL
            gfull_d = [[dr.tile([RW, 128], fp16, tag=f"gfull{l}_{k}",
                                addr_space="Shared" if USE_SHARED else "Local",
                                name=f"gfull{l}_{k}_d")
                        for k in range(NR)]
                       for l in range(3)]

            x3e = pp.tile([f_out, nt * P], fp16, tag="yT0")
            g_loc = pp.tile([P, nt, f_hid], fp16, tag="gloc")
            nc.vector.memset(g_loc[:, :, :], 0.0)

            qload = [0] * NQ
            RWIDE = N_CORES * SL  # rows per slice region of gfull

            last_gather = [None]

            def gather_call(layer, cls, lo, nb, fo):
                g_t = gp.tile([P, maxb, fo], fp16, tag="G", name="g_t")
                if cls == 0:
                    in_ap = shard_d[layer][0:npc, 0:fo]
                else:
                    in_ap = gfull_d[layer][cls - 1][0:RWIDE, 0:fo]
                qn = min(range(NQ), key=lambda i: qload[i])
                qload[qn] += nb
                last_gather[0] = dma_gather_raw(
                    nc.gpsimd,
                    out_ap=g_t[:, 0:nb, :],
                    in_ap=in_ap,
                    idxs_ap=idx_sb[:, lo * 8: (lo + nb) * 8],
                    num_idxs=nb * P,
                    elem_size=fo,
                    elem_step=128,
                    queue_num=qn,
                    single_packet=SINGLE_PKT,
                )
                return g_t

            def h_tile(layer, t):
                """h = y_prev @ W for one node tile; dis-scaled fp16 copy to
                g_loc, then DMA to this layer's shard buffer."""
                fi, fo = fdims[layer]
                tw = min(P, npc - t * P)
                if layer == 0:
                    lhsT = xT_sb[:, t * P: t * P + tw]
                else:
                    lhsT = yT[(layer + 1) % 2][:fi, t * P: t * P + tw]
                pg = psB.tile([P, fo], fp32, tag="pg", space="PSUM")
                nc.tensor.matmul(pg[:tw, :], lhsT=lhsT, rhs=W_sb[layer][:],
                                 start=True, stop=True)
                gsl = g_loc[:, t, 0:fo]
                nc.vector.tensor_scalar(
                    out=gsl[:tw, :], in0=pg[:tw, :],
                    scalar1=dis_sb[:tw, t: t + 1], scalar2=None,
                    op0=mybir.AluOpType.mult,
                )
                nc.sync.dma_start(shard_d[layer][t * P: t * P + tw, 0:fo],
                                  gsl[:tw, :])

            def emit_AG(layer, k, after=None):
                """sub-AG k of `layer` ships every core's rows [k*SL,(k+1)*SL)
                of that layer's shard into the shared slice-k region.
                Collectives serialize end-to-start on one cc stream AND hold
                gpsimd's in-order queue while their wait is pending, so each
                trigger is placed where its wait is already satisfied.
                `after` pins the trigger behind a gather instruction so the
                scheduler cannot hoist it to the head of gpsimd's queue."""
                cc = nc.gpsimd.collective_compute(
                    "AllGather",
                    mybir.AluOpType.bypass,
                    replica_groups=[list(range(N_CORES))],
                    ins=[shard_d[layer][k * SL: (k + 1) * SL, :]],
                    outs=[gfull_d[layer][k][0:RWIDE, :]],
                )
                if after is not None:
                    tile.add_dep_helper(
                        cc.ins, after.ins,
                        sync=True, reason="AG behind early local gathers")
                return cc

            # layer-0 h-phase; later layers' h tiles are emitted inside the
            # previous layer's chain loop (pipelines the layer boundary)
            for t in range(nt):
                h_tile(0, t)

            for layer in range(3):
                fi, fo = fdims[layer]
                # ---- gather stream, interleaved with this layer's AG
                # triggers (layer 0 only; later layers' AGs are emitted
                # inline in the previous layer's chain loop): the trigger of
                # AG k sits behind enough gather work that AG k-1 has
                # completed, so it never stalls gpsimd ----
                # layer 0: locals, then AG k before the first class-(k+1)
                # call.  Layers 1-2: all AGs were triggered inline in the
                # previous layer's chain loop as each shard slice completed,
                # so they land before this layer's slice gathers need them.
                Gt = {}  # (cls, lo) -> tile (keyed by call)
                for (q, lo, nb, dummy) in struct["calls"]:
                    if q == 0:
                        Gt[(q, lo)] = gather_call(layer, q, lo, nb, fo)
                next_ag = 0
                for (q, lo, nb, dummy) in struct["calls"]:
                    if q == 0:
                        continue
                    if layer == 0:
                        while next_ag < NR and q > next_ag:
                            emit_AG(0, next_ag)
                            next_ag += 1
                    g_t = gather_call(layer, q, lo, nb, fo)
                    if not dummy:
                        Gt[(q, lo)] = g_t

                # map global block id -> (tile handle, block within call)
                chunk_tile = {}
                for (q, lo, nb, dummy) in struct["calls"]:
                    if dummy:
                        continue
                    for b in range(nb):
                        chunk_tile[lo + b] = (Gt[(q, lo)], b)

                # ---- per-tile accumulation chains (variable-K pieces) ----
                for t in range(nt):
                    tw = min(P, npc - t * P)
                    cot = struct["pieces_of_tile"][t]
                    ncot = len(cot)
                    wS = (ncot + 1) * P
                    # S loads alternate between the Scalar and Sync DMA
                    # queues: one queue serializes the ~650KB/tile S stream
                    # and ends up pacing the whole chain pipeline
                    S_tbuf = selp.tile([P, maxcpt * P], fp16, tag="St")
                    s_eng = nc.scalar if t % 2 == 0 else nc.sync
                    s_eng.dma_start(
                        S_tbuf[:, 0:wS],
                        S_in[:, tile_off[t] * P: tile_off[t + 1] * P])
                    pa = psA.tile([fo, P], fp32, tag="pa", space="PSUM")
                    for j, (g, ro, K, q) in enumerate(cot):
                        g_t, blk = chunk_tile[g]
                        # PE operands must start at partition 0/32/64; use
                        # rows [0, ro+K) — S rows below ro are zero for this
                        # piece's columns, so the extra rows contribute 0
                        nc.tensor.matmul(
                            pa[:, :],
                            lhsT=g_t[0: ro + K, blk, :],
                            rhs=S_tbuf[0: ro + K, j * P: (j + 1) * P],
                            start=(j == 0),
                            stop=False,
                        )
                    # diag (self-loop) chunk: lhsT = local activations
                    nc.tensor.matmul(
                        pa[:, :],
                        lhsT=g_loc[:, t, 0:fo],
                        rhs=S_tbuf[:, ncot * P: wS],
                        start=False,
                        stop=True,
                    )
                    if layer < 2:
                        nc.scalar.activation(
                            out=yT[layer % 2][:fo, t * P: t * P + tw],
                            in_=pa[:, :tw],
                            func=mybir.ActivationFunctionType.Relu,
                            bias=b_sb[layer][:, :1],
                            scale=1.0,
                        )
                        # next layer's h for this tile, right behind the
                        # epilogue: the next shard fills as chains drain, and
                        # each next-layer sub-AG fires as soon as its slice
                        # of the shard is complete (its wait is satisfied at
                        # that point, so it never stalls gpsimd's queue)
                        h_tile(layer + 1, t)
                        # inline-trigger only the sub-AGs whose shard slices
                        # complete; each fires as its shard slice finishes
                        for k in range(NR):
                            if t == ((k + 1) * SL + P - 1) // P - 1:
                                emit_AG(layer + 1, k)
                    else:
                        nc.scalar.activation(
                            out=x3e[:, t * P: t * P + tw],
                            in_=pa[:, :tw],
                            func=mybir.ActivationFunctionType.Exp,
                            bias=b_sb[2][:, :1],
                            scale=1.0,
                        )

            # ---- log_softmax tail: out = ln(e) - ln(sum_part(e)) ----
            W3T = 512
            for o in range(0, npc, W3T):
                wdt = min(W3T, npc - o)
                ps_s = psC.tile([1, W3T], fp32, tag="l3s", space="PSUM")
                nc.tensor.matmul(ps_s[:1, :wdt], lhsT=ones_col[:],
                                 rhs=x3e[:, o: o + wdt], start=True, stop=True)
                ls_t = wp.tile([1, W3T], fp32, tag="ls")
                nc.scalar.activation(
                    out=ls_t[:1, :wdt], in_=ps_s[:1, :wdt],
                    func=mybir.ActivationFunctionType.Ln, bias=0.0, scale=1.0,
                )
                nc.scalar.activation(
                    out=x3e[:, o: o + wdt], in_=x3e[:, o: o + wdt],
                    func=mybir.ActivationFunctionType.Ln, bias=0.0, scale=1.0,
                )
                ps_b = psC.tile([f_out, W3T], fp32, tag="l3b", space="PSUM")
                nc.tensor.matmul(ps_b[:, :wdt], lhsT=ones_row[:],
                                 rhs=ls_t[:1, :wdt], start=True, stop=True)
                o_sb = wp.tile([f_out, W3T], fp32, tag="o3")
                nc.vector.tensor_tensor(
                    out=o_sb[:, :wdt], in0=x3e[:, o: o + wdt],
                    in1=ps_b[:, :wdt], op=mybir.AluOpType.subtract,
                )
                nc.sync.dma_start(out3T[:, o: o + wdt], o_sb[:, :wdt])

    nc.compile()
    return nc


# ----------------------------------------------------------------- kernel ---
_CACHE = {}


def kernel(x, edge_index, W1, b1, W2, b2, W3, b3):
    global LAST_RESULT
    x = np.asarray(x)
    edge_index = np.asarray(edge_index)
    n, f_in = x.shape
    f_hid = np.asarray(W2).shape[0]
    f_out = np.asarray(W3).shape[1]
    assert n % N_CORES == 0
    npc = n // N_CORES

    pkey = (edge_index.shape, int(edge_index[0, 0]), int(edge_index[1, -1]),
            int(edge_index[0].sum() % (1 << 62)))
    hit = _CACHE.get(("prep", pkey))
    if hit is None:
        hit = _prepare_spmd(edge_index, n, npc)
        _CACHE[("prep", pkey)] = hit
    struct, per_core = hit

    ckey = (n, f_in, f_hid, f_out, struct["nchunk_g"], struct["max_nb"],
            tuple(struct["ranges"]))
    if ckey not in _CACHE:
        _CACHE[ckey] = _build(struct, n, npc, f_in, f_hid, f_out)
    nc = _CACHE[ckey]

    ones40 = np.ones((f_out, 1), np.float32)
    in_maps = []
    for c in range(N_CORES):
        pc = per_core[c]
        in_maps.append({
            "xT": np.ascontiguousarray(x[c * npc: (c + 1) * npc].T).astype(np.float16),
            "W1": np.asarray(W1, np.float16), "b1": np.asarray(b1, np.float32).reshape(-1, 1),
            "W2": np.asarray(W2, np.float16), "b2": np.asarray(b2, np.float32).reshape(-1, 1),
            "W3": np.asarray(W3, np.float16),
            # -8 shift: log_softmax is shift-invariant; keeps fp16 exp in range
            "b3": np.asarray(b3, np.float32).reshape(-1, 1) - 8.0,
            "idx_all": pc["idx16"], "S_all": pc["S"], "dis_col": pc["dis_col"],
            "ones40": ones40, "ones40h": ones40.astype(np.float16),
        })
    kw = {}
    if TRACE:
        import tempfile
        kw = dict(trace=True, trace_cores=[0],
                  tmpdir=tempfile.mkdtemp(prefix="gcn_v3_"))
    res = run_bass_kernel_spmd(nc, in_maps, core_ids=list(range(N_CORES)), **kw)
    LAST_RESULT = res
    out = np.concatenate(
        [res.results[c]["out3T"].T for c in range(N_CORES)], axis=0
    ).astype(np.float32)
    return out


# revision 85
# speedup vs baseline: 1.0153x; 1.0153x over previous
"""GCN (3-layer, PyG GCNConv semantics) on 8 Trainium2 NeuronCores.

Strategy (graph/data parallel, dst-sharded), v3:
  - Nodes sharded across 8 cores (rows of x / output).
  - Per layer: each core computes its slice of h = y_prev @ W on PE,
    scales rows by dis[node] (1/sqrt(deg), folded into the PSUM->SBUF
    copy as a per-partition tensor_scalar mult), writes it (fp16,
    256B-strided rows) to a per-layer shard buffer, then sub-AllGathers
    into a per-layer SHARED-scratchpad gfull table (write-once across
    the 8 cores instead of 8 per-core copies).
  - Aggregation: edges bucketed by (dst tile, src class); chunks of 128
    edges; dma_gather (custom 128B-payload lowering) fetches g[src]
    rows in large calls (up to NBMAX blocks per call; the SWDGE
    descriptor ring is enlarged via dynamic_dma_scratch_size to allow
    ~1792 rows/call, amortizing the ~1us fixed descriptor-gen cost).
  - The scatter-add becomes PE matmuls agg_T += G.T @ S where S is a
    HOST-PRECOMPUTED one-hot whose values are dis[dst] (so that
    message = dis[src]*h[src] matmul S gives the symmetric norm);
    S lives in DRAM and is DMA'd per tile per layer (no vector build).
    Self-loops are a diag chunk whose lhsT is the local (dis-scaled)
    activation tile and whose S diag is also dis[dst].
  - Epilogue: relu(agg + b) in one ScalarE activation (transposed
    layout: bias is per-partition). Final layer: log_softmax via exp
    (ACT), partition-sum (PE ones-matmul), ln (ACT), broadcast (PE),
    subtract.
  - Output is produced transposed [40, nodes] per core; host transposes.

Self-contained: only needs numpy + the concourse stack at /opt/trn_rl_repo.
"""

import os
import sys

sys.path.insert(0, "/opt/trn_rl_repo")

import numpy as np

import concourse.bacc as bacc
import concourse.tile as tile
import concourse.mybir as mybir
from concourse import ap_utils
from concourse.bass import AP, MemorySpace
from concourse.bass_utils import run_bass_kernel_spmd

fp32 = mybir.dt.float32
fp16 = mybir.dt.float16
i16 = mybir.dt.int16

N_CORES = 8
P = 128
NBMAX = int(os.environ.get("GCN_NBMAX", "8"))
DMA_SCRATCH = int(os.environ.get("GCN_SCRATCH", "16384"))
SINGLE_PKT = os.environ.get("GCN_SP", "1") == "1"
# max slice rows: N_CORES*SL must fit the int16 gather-index window (32768).
# Kept well under the 4095 cap: smaller sub-AGs shrink the end-to-start
# transit of the FINAL sub-AG, which is exactly the layer-boundary stall.
SL_CAP = int(os.environ.get("GCN_SLCAP", "2500"))
NQ = 4             # SWDGE queues

# test.py sets this to capture profile info; harness leaves it off.
TRACE = os.environ.get("GCN_TRACE", "0") == "1"
USE_SHARED = os.environ.get("GCN_SHARED", "1") == "1"
LAST_RESULT = None


# ---------------------------------------------------------------- gather ----
def dma_gather_raw(engine, out_ap, in_ap, idxs_ap, num_idxs, elem_size, elem_step,
                   queue_num=0, single_packet=True):
    """bass dma_gather minus the elem_size%256B assert: the ucode only needs
    the row STRIDE 256B-quantized; the payload is free."""
    assert idxs_ap.dtype == mybir.dt.int16
    assert in_ap.space == MemorySpace.DRAM
    assert out_ap.space == MemorySpace.SBUF
    assert in_ap.dtype == out_ap.dtype
    assert ap_utils.ap_is_contiguous(out_ap.ap[1:])
    assert ap_utils.ap_is_contiguous(idxs_ap.ap[1:])
    assert in_ap.ap[-1][1] == elem_size
    assert out_ap.ap[-1][1] == elem_size
    assert in_ap.ap[0][0] == elem_step
    stride_bytes = elem_step * mybir.dt.size(in_ap.dtype)
    assert stride_bytes % 256 == 0
    return engine.add_instruction(
        mybir.InstDMAGatherAnt(
            name=engine.bass.get_next_instruction_name(),
            ins=[
                *engine.lower_ap_dma(in_ap, for_custom_bir_dma=True),
                engine.lower_ap(idxs_ap),
                engine.lower_val_access(engine.to_reg(num_idxs)),
            ],
            outs=[engine.lower_ap(out_ap)],
            transpose=False,
            num_idxs=num_idxs,
            elem_size=elem_size,
            stride_bytes_256=stride_bytes // 256,
            gen_mode=0,
            single_packet=single_packet,
            queue_num=queue_num,
            sbuf_tokens_per_rank=0,
            sbuf_free_dim_per_rank=0,
            sbuf_free_dim_pad_per_rank=0,
            sbuf_byte_offset=0,
        )
    )


# ---------------------------------------------------------- host preprocess --
def _prepare_spmd(edge_index, n, npc):
    """Shared (max-over-cores) chunk layout so all cores run one program.

    Src classes: 0 = local (own shard, gathered from the per-layer shard
    buffer pre-collective), 1+k = slice k of gfull.  The AllGather is split
    into NR sub-collectives; sub-AG k gathers every core's local rows
    [k*SL, (k+1)*SL) into a contiguous gfull region of N_CORES*SL rows
    ordered (core, row%SL), so slice-k gathers can start as soon as
    sub-AG k lands (int16 gather indices are region-relative).

    Also host-builds the per-core S tables: for each chunk, S[slot, d] =
    dis[dst] at the edge's dst-local column (one-hot rows), plus a diag
    chunk per tile with S[d, d] = dis[dst].  Messages are dis[src]-scaled
    on-chip, so G.T @ S accumulates dis[src]*dis[dst]*h[src] per dst.
    """
    src = np.asarray(edge_index[0], np.int64)
    dst = np.asarray(edge_index[1], np.int64)
    deg = np.bincount(dst, minlength=n).astype(np.float64) + 1.0
    dis64 = 1.0 / np.sqrt(deg)
    dis = dis64.astype(np.float32)

    # dedup repeated (s, d) edges (randint graphs have ~6%): a repeated edge
    # contributes m * dis[s] * dis[d] * h[s]; fold m into the S value
    eid = src * n + dst
    uniq, ucnt = np.unique(eid, return_counts=True)
    src = uniq // n
    dst = uniq % n
    emult = ucnt.astype(np.float32)

    NR = (npc + SL_CAP - 1) // SL_CAP
    while npc % NR != 0 and NR < npc:
        NR += 1
    SL = npc // NR
    n_tiles = (npc + P - 1) // P
    NCLS = 1 + NR

    # per-core bucket counts
    cnts = np.zeros((N_CORES, n_tiles, NCLS), np.int64)
    core_edges = []
    for c in range(N_CORES):
        m = (dst >= c * npc) & (dst < (c + 1) * npc)
        s_c, d_c, w_c = src[m], dst[m] - c * npc, emult[m]
        tile_c = d_c // P
        own = (s_c >= c * npc) & (s_c < (c + 1) * npc)
        src_core = s_c // npc
        src_loc = s_c - src_core * npc
        slc = src_loc // SL
        cls = np.where(own, 0, 1 + slc)
        rel = np.where(own, src_loc,
                       src_core * SL + (src_loc - slc * SL))
        order = np.lexsort((s_c, cls, tile_c))
        tile_s, cls_s = tile_c[order], cls[order]
        bucket = tile_s * NCLS + cls_s
        cnts[c] = np.bincount(bucket, minlength=n_tiles * NCLS).reshape(
            n_tiles, NCLS)
        core_edges.append((bucket, rel[order], (d_c[order] - tile_s * P),
                           w_c[order]))

    # --- zero-ceil piece packing ---------------------------------------
    # Per class, runs (t, q) of cnt_max = max-over-core edges are laid out
    # back-to-back in one slot stream; 128-slot blocks are the gather /
    # matmul-lhsT unit, and a run crossing a block boundary is split into
    # PIECES (variable-K matmuls).  No per-(tile,class) ceil padding: only
    # the SPMD max-over-cores spread and per-class call tails pad.
    cnt_max = cnts.max(axis=0)  # [n_tiles, NCLS]
    block_base = np.zeros(NCLS + 1, np.int64)   # global block id per class
    run_start = np.zeros((n_tiles, NCLS), np.int64)  # class-local slot start
    g = 0
    for q in range(NCLS):
        block_base[q] = g
        pos = 0
        for t in range(n_tiles):
            run_start[t, q] = pos
            pos += int(cnt_max[t, q])
        g += (pos + P - 1) // P
    block_base[NCLS] = g
    nchunk_g = g

    # pieces per tile: (block, row_off, K, q), in class order then slot order
    pieces_of_tile = [[] for _ in range(n_tiles)]
    run_piece_base = np.zeros((n_tiles, NCLS), np.int64)
    block_tile_of = np.full(max(nchunk_g, 1), -1, np.int64)
    for q in range(NCLS):
        for t in range(n_tiles):
            s0 = int(run_start[t, q])
            cntq = int(cnt_max[t, q])
            run_piece_base[t, q] = len(pieces_of_tile[t])
            while cntq > 0:
                ro = s0 % P
                K = min(P - ro, cntq)
                blk = int(block_base[q] + s0 // P)
                if block_tile_of[blk] < 0:
                    block_tile_of[blk] = t  # first tile served by this block
                pieces_of_tile[t].append((blk, int(ro), int(K), q))
                s0 += K
                cntq -= K
    block_tile_of[block_tile_of < 0] = 0

    # calls: per class, maximally-full NBMAX-block runs; local class first,
    # then slice calls sorted by (first tile served, class) so gpsimd's
    # in-order stream matches tile-major consumption
    def class_calls(q):
        out = []
        lo = int(block_base[q])
        hi = int(block_base[q + 1])
        while lo < hi:
            nb = min(NBMAX, hi - lo)
            out.append((q, lo, nb, False))
            lo += nb
        return out

    calls = class_calls(0)
    n_local_calls = len(calls)
    slice_calls = []
    for q in range(1, NCLS):
        slice_calls.extend(class_calls(q))
    slice_calls.sort(key=lambda c: (int(block_tile_of[c[1]]), c[0]))
    assert all(not c[3] for c in calls + slice_calls)
    calls = calls + slice_calls
    ranges = []

    # S layout: per tile, one 128-col block per piece, then the diag block
    tile_off = np.zeros(n_tiles + 1, np.int64)
    for t in range(n_tiles):
        tile_off[t + 1] = tile_off[t] + len(pieces_of_tile[t]) + 1
    nchunk_all = int(tile_off[n_tiles])

    per_core = []
    for c in range(N_CORES):
        bucket, rel_s, dl_s, w_s = core_edges[c]
        cnt = cnts[c].reshape(-1)
        bs = np.concatenate([[0], np.cumsum(cnt)[:-1]])
        rank = np.arange(len(bucket)) - bs[bucket]
        t_of = bucket // NCLS
        q_of = bucket % NCLS
        # class-local slot of this edge, then global gather slot via blocks
        cslot = run_start[t_of, q_of] + rank
        gslot = block_base[q_of] * P + cslot
        slot = gslot % P

        idx_flat = np.zeros(max(nchunk_g, 1) * P, np.int64)
        idx_flat[gslot] = rel_s

        # host-built S table [P, nchunk_all*P] fp16: edge's piece within its
        # run = how many block boundaries its class-slot has crossed
        S = np.zeros((P, nchunk_all * P), np.float16)
        piece_idx = run_piece_base[t_of, q_of] + \
            (cslot // P - run_start[t_of, q_of] // P)
        ecol = (tile_off[t_of] + piece_idx) * P + dl_s
        vals = (w_s * dis[c * npc + t_of * P + dl_s]).astype(np.float16)
        S[slot, ecol] = vals
        # diag blocks
        for t in range(n_tiles):
            tw = min(P, npc - t * P)
            dcol = (tile_off[t] + len(pieces_of_tile[t])) * P
            dd = np.arange(tw)
            S[dd, dcol + dd] = dis[c * npc + t * P + dd].astype(np.float16)

        # dis column table [P, n_tiles] fp32 (per-partition scale in h_tile)
        dis_col = np.zeros((P, n_tiles), np.float32)
        node = c * npc + np.arange(npc)
        dis_col[np.arange(npc) % P, np.arange(npc) // P] = dis[node]

        tmp = idx_flat.astype(np.int16).reshape(max(nchunk_g, 1) * 8, 16).T
        idx16 = np.tile(np.ascontiguousarray(tmp), (8, 1))
        per_core.append(dict(idx16=idx16, S=S, dis_col=dis_col))

    struct = dict(n_tiles=n_tiles, NR=NR, SL=SL, ranges=ranges, calls=calls,
                  nchunk_g=nchunk_g, nchunk_all=nchunk_all,
                  pieces_of_tile=pieces_of_tile, tile_off=tile_off,
                  n_local_calls=n_local_calls,
                  max_nb=max(c[2] for c in calls) if calls else 1)
    return struct, per_core


# ----------------------------------------------------------------- program --
def _build(struct, n, npc, f_in, f_hid, f_out):
    nt = struct["n_tiles"]
    NR = struct["NR"]
    SL = struct["SL"]
    nchunk_g = struct["nchunk_g"]
    nchunk_all = struct["nchunk_all"]
    tile_off = struct["tile_off"]
    maxb = struct["max_nb"]
    maxcpt = max(len(c) for c in struct["pieces_of_tile"]) + 1
    fdims = [(f_in, f_hid), (f_hid, f_hid), (f_hid, f_out)]
    ic = max(nchunk_g, 1) * 8

    nc = bacc.Bacc("TRN2", target_bir_lowering=False, debug=False,
                   num_devices=N_CORES, num_swdge_queues=NQ,
                   dynamic_dma_scratch_size=DMA_SCRATCH)
    xT = nc.dram_tensor("xT", [f_in, npc], fp16, kind="ExternalInput").ap()
    Ws = [nc.dram_tensor(f"W{i+1}", [fi, fo], fp16, kind="ExternalInput").ap()
          for i, (fi, fo) in enumerate(fdims)]
    bs = [nc.dram_tensor(f"b{i+1}", [fo, 1], fp32, kind="ExternalInput").ap()
          for i, (_, fo) in enumerate(fdims)]
    idx_in = nc.dram_tensor("idx_all", [P, ic], i16, kind="ExternalInput").ap()
    S_in = nc.dram_tensor("S_all", [P, nchunk_all * P], fp16,
                          kind="ExternalInput").ap()
    dis_in = nc.dram_tensor("dis_col", [P, nt], fp32, kind="ExternalInput").ap()
    ones_in = nc.dram_tensor("ones40", [f_out, 1], fp32, kind="ExternalInput").ap()
    ones16_in = nc.dram_tensor("ones40h", [f_out, 1], fp16, kind="ExternalInput").ap()
    out3T = nc.dram_tensor("out3T", [f_out, npc], fp32, kind="ExternalOutput").ap()

    with tile.TileContext(nc) as tc:
        with (
            tc.tile_pool(name="const", bufs=1) as cp,
            tc.tile_pool(name="gather", bufs=struct["n_local_calls"] + 20) as gp,
            tc.tile_pool(name="sel", bufs=8) as selp,
            tc.tile_pool(name="work", bufs=3) as wp,
            tc.tile_pool(name="persist", bufs=1) as pp,
            tc.tile_pool(name="psA", bufs=3, space="PSUM") as psA,
            tc.tile_pool(name="psB", bufs=1, space="PSUM") as psB,
            tc.tile_pool(name="psC", bufs=2, space="PSUM") as psC,
            tc.tile_pool(name="dram", bufs=1, space="DRAM") as dr,
        ):
            # idx table split-loaded so the first gather calls only wait for
            # their own slice of the 2.6MB table, not the whole DMA
            idx_sb = cp.tile([P, ic], i16)
            ISPL = max(1, ic // 8)
            for o in range(0, ic, ISPL):
                w = min(ISPL, ic - o)
                nc.sync.dma_start(idx_sb[:, o: o + w], idx_in[:, o: o + w])
            dis_sb = cp.tile([P, nt], fp32)
            nc.sync.dma_start(dis_sb[:], dis_in[:])
            W_sb = []
            b_sb = []
            for i, (fi, fo) in enumerate(fdims):
                wt = cp.tile([fi, fo], fp16, tag=f"W{i}")
                nc.sync.dma_start(wt[:], Ws[i][:])
                W_sb.append(wt[:])
                b = cp.tile([fo, 1], fp32, tag=f"b{i}")
                nc.sync.dma_start(b[:], bs[i][:])
                b_sb.append(b)
            ones_col = cp.tile([f_out, 1], fp16)
            nc.sync.dma_start(ones_col[:], ones16_in[:])
            ones_row = cp.tile([1, f_out], fp32)
            nc.sync.dma_start(ones_row[:], ones_in[:].transpose([1, 0]))

            # split the xT load so early h tiles don't wait for the full 3.2MB
            xT_sb = pp.tile([f_in, npc], fp16, tag="xT")
            XSPLIT = max(1, nt // 8)
            for o in range(0, npc, XSPLIT * P):
                w = min(XSPLIT * P, npc - o)
                nc.sync.dma_start(xT_sb[:, o: o + w], xT[:, o: o + w])
            yT0 = pp.tile([f_hid, nt * P], fp16, tag="yT0")
            yT1 = pp.tile([f_hid, nt * P], fp16, tag="yT1")
            yT = [yT0[:], yT1[:]]

            # per-layer shard + shared gfull (no cross-layer WAR at all)
            shard_d = [dr.tile([npc, 128], fp16, tag=f"shard{l}",
                               name=f"shard{l}_d")
                       for l in range(3)]
            # one shared tensor per (layer, slice region): the tile framework
            # allows only a single writing instruction per Shared DRAM tensor
            RW = N_CORES * SL
            gfull_d = [[dr.tile([RW, 128], fp16, tag=f"gfull{l}_{k}",
                                addr_space="Shared" if USE_SHARED else "Local",
                                name=f"gfull{l}_{k}_d")
                        for k in range(NR)]
                       for l in range(3)]

            x3e = pp.tile([f_out, nt * P], fp16, tag="yT0")
            g_loc = pp.tile([P, nt, f_hid], fp16, tag="gloc")
            nc.vector.memset(g_loc[:, :, :], 0.0)

            qload = [0] * NQ
            RWIDE = N_CORES * SL  # rows per slice region of gfull

            last_gather = [None]

            def gather_call(layer, cls, lo, nb, fo):
                g_t = gp.tile([P, maxb, fo], fp16, tag="G", name="g_t")
                if cls == 0:
                    in_ap = shard_d[layer][0:npc, 0:fo]
                else:
                    in_ap = gfull_d[layer][cls - 1][0:RWIDE, 0:fo]
                qn = min(range(NQ), key=lambda i: qload[i])
                qload[qn] += nb
                last_gather[0] = dma_gather_raw(
                    nc.gpsimd,
                    out_ap=g_t[:, 0:nb, :],
                    in_ap=in_ap,
                    idxs_ap=idx_sb[:, lo * 8: (lo + nb) * 8],
                    num_idxs=nb * P,
                    elem_size=fo,
                    elem_step=128,
                    queue_num=qn,
                    single_packet=SINGLE_PKT,
                )
                return g_t

            def h_tile(layer, t):
                """h = y_prev @ W for one node tile; dis-scaled fp16 copy to
                g_loc, then DMA to this layer's shard buffer."""
                fi, fo = fdims[layer]
                tw = min(P, npc - t * P)
                if layer == 0:
                    lhsT = xT_sb[:, t * P: t * P + tw]
                else:
                    lhsT = yT[(layer + 1) % 2][:fi, t * P: t * P + tw]
                pg = psB.tile([P, fo], fp32, tag="pg", space="PSUM")
                nc.tensor.matmul(pg[:tw, :], lhsT=lhsT, rhs=W_sb[layer][:],
                                 start=True, stop=True)
                gsl = g_loc[:, t, 0:fo]
                nc.vector.tensor_scalar(
                    out=gsl[:tw, :], in0=pg[:tw, :],
                    scalar1=dis_sb[:tw, t: t + 1], scalar2=None,
                    op0=mybir.AluOpType.mult,
                )
                nc.sync.dma_start(shard_d[layer][t * P: t * P + tw, 0:fo],
                                  gsl[:tw, :])

            def emit_AG(layer, k, after=None):
                """sub-AG k of `layer` ships every core's rows [k*SL,(k+1)*SL)
                of that layer's shard into the shared slice-k region.
                Collectives serialize end-to-start on one cc stream AND hold
                gpsimd's in-order queue while their wait is pending, so each
                trigger is placed where its wait is already satisfied.
                `after` pins the trigger behind a gather instruction so the
                scheduler cannot hoist it to the head of gpsimd's queue."""
                cc = nc.gpsimd.collective_compute(
                    "AllGather",
                    mybir.AluOpType.bypass,
                    replica_groups=[list(range(N_CORES))],
                    ins=[shard_d[layer][k * SL: (k + 1) * SL, :]],
                    outs=[gfull_d[layer][k][0:RWIDE, :]],
                )
                if after is not None:
                    tile.add_dep_helper(
                        cc.ins, after.ins,
                        sync=True, reason="AG behind early local gathers")
                return cc

            # layer-0 h-phase; later layers' h tiles are emitted inside the
            # previous layer's chain loop (pipelines the layer boundary)
            for t in range(nt):
                h_tile(0, t)

            for layer in range(3):
                fi, fo = fdims[layer]
                # ---- gather stream, interleaved with this layer's AG
                # triggers (layer 0 only; later layers' AGs are emitted
                # inline in the previous layer's chain loop): the trigger of
                # AG k sits behind enough gather work that AG k-1 has
                # completed, so it never stalls gpsimd ----
                # layer 0: locals, then AG k before the first class-(k+1)
                # call.  Layers 1-2: all AGs were triggered inline in the
                # previous layer's chain loop as each shard slice completed,
                # so they land before this layer's slice gathers need them.
                Gt = {}  # (cls, lo) -> tile (keyed by call)
                for (q, lo, nb, dummy) in struct["calls"]:
                    if q == 0:
                        Gt[(q, lo)] = gather_call(layer, q, lo, nb, fo)
                next_ag = 0
                for (q, lo, nb, dummy) in struct["calls"]:
                    if q == 0:
                        continue
                    if layer == 0:
                        while next_ag < NR and q > next_ag:
                            emit_AG(0, next_ag)
                            next_ag += 1
                    g_t = gather_call(layer, q, lo, nb, fo)
                    if not dummy:
                        Gt[(q, lo)] = g_t

                # map global block id -> (tile handle, block within call)
                chunk_tile = {}
                for (q, lo, nb, dummy) in struct["calls"]:
                    if dummy:
                        continue
                    for b in range(nb):
                        chunk_tile[lo + b] = (Gt[(q, lo)], b)

                # ---- per-tile accumulation chains (variable-K pieces) ----
                for t in range(nt):
                    tw = min(P, npc - t * P)
                    cot = struct["pieces_of_tile"][t]
                    ncot = len(cot)
                    wS = (ncot + 1) * P
                    # S loads alternate between the Scalar and Sync DMA
                    # queues: one queue serializes the ~650KB/tile S stream
                    # and ends up pacing the whole chain pipeline
                    S_tbuf = selp.tile([P, maxcpt * P], fp16, tag="St")
                    s_eng = nc.scalar if t % 2 == 0 else nc.sync
                    s_eng.dma_start(
                        S_tbuf[:, 0:wS],
                        S_in[:, tile_off[t] * P: tile_off[t + 1] * P])
                    pa = psA.tile([fo, P], fp32, tag="pa", space="PSUM")
                    for j, (g, ro, K, q) in enumerate(cot):
                        g_t, blk = chunk_tile[g]
                        # PE operands must start at partition 0/32/64; use
                        # rows [0, ro+K) — S rows below ro are zero for this
                        # piece's columns, so the extra rows contribute 0
                        nc.tensor.matmul(
                            pa[:, :],
                            lhsT=g_t[0: ro + K, blk, :],
                            rhs=S_tbuf[0: ro + K, j * P: (j + 1) * P],
                            start=(j == 0),
                            stop=False,
                        )
                    # diag (self-loop) chunk: lhsT = local activations
                    nc.tensor.matmul(
                        pa[:, :],
                        lhsT=g_loc[:, t, 0:fo],
                        rhs=S_tbuf[:, ncot * P: wS],
                        start=False,
                        stop=True,
                    )
                    if layer < 2:
                        nc.scalar.activation(
                            out=yT[layer % 2][:fo, t * P: t * P + tw],
                            in_=pa[:, :tw],
                            func=mybir.ActivationFunctionType.Relu,
                            bias=b_sb[layer][:, :1],
                            scale=1.0,
                        )
                        # next layer's h for this tile, right behind the
                        # epilogue: the next shard fills as chains drain, and
                        # each next-layer sub-AG fires as soon as its slice
                        # of the shard is complete (its wait is satisfied at
                        # that point, so it never stalls gpsimd's queue)
                        h_tile(layer + 1, t)
                        # inline-trigger only the sub-AGs whose shard slices
                        # complete; each fires as its shard slice finishes
                        for k in range(NR):
                            if t == ((k + 1) * SL + P - 1) // P - 1:
                                emit_AG(layer + 1, k)
                    else:
                        nc.scalar.activation(
                            out=x3e[:, t * P: t * P + tw],
                            in_=pa[:, :tw],
                            func=mybir.ActivationFunctionType.Exp,
                            bias=b_sb[2][:, :1],
                            scale=1.0,
                        )

            # ---- log_softmax tail: out = ln(e) - ln(sum_part(e)) ----
            W3T = 512
            for o in range(0, npc, W3T):
                wdt = min(W3T, npc - o)
                ps_s = psC.tile([1, W3T], fp32, tag="l3s", space="PSUM")
                nc.tensor.matmul(ps_s[:1, :wdt], lhsT=ones_col[:],
                                 rhs=x3e[:, o: o + wdt], start=True, stop=True)
                ls_t = wp.tile([1, W3T], fp32, tag="ls")
                nc.scalar.activation(
                    out=ls_t[:1, :wdt], in_=ps_s[:1, :wdt],
                    func=mybir.ActivationFunctionType.Ln, bias=0.0, scale=1.0,
                )
                nc.scalar.activation(
                    out=x3e[:, o: o + wdt], in_=x3e[:, o: o + wdt],
                    func=mybir.ActivationFunctionType.Ln, bias=0.0, scale=1.0,
                )
                ps_b = psC.tile([f_out, W3T], fp32, tag="l3b", space="PSUM")
                nc.tensor.matmul(ps_b[:, :wdt], lhsT=ones_row[:],
                                 rhs=ls_t[:1, :wdt], start=True, stop=True)
                o_sb = wp.tile([f_out, W3T], fp32, tag="o3")
                nc.vector.tensor_tensor(
                    out=o_sb[:, :wdt], in0=x3e[:, o: o + wdt],
                    in1=ps_b[:, :wdt], op=mybir.AluOpType.subtract,
                )
                nc.sync.dma_start(out3T[:, o: o + wdt], o_sb[:, :wdt])

    nc.compile()
    return nc


# ----------------------------------------------------------------- kernel ---
_CACHE = {}


def kernel(x, edge_index, W1, b1, W2, b2, W3, b3):
    global LAST_RESULT
    x = np.asarray(x)
    edge_index = np.asarray(edge_index)
    n, f_in = x.shape
    f_hid = np.asarray(W2).shape[0]
    f_out = np.asarray(W3).shape[1]
    assert n % N_CORES == 0
    npc = n // N_CORES

    pkey = (edge_index.shape, int(edge_index[0, 0]), int(edge_index[1, -1]),
            int(edge_index[0].sum() % (1 << 62)))
    hit = _CACHE.get(("prep", pkey))
    if hit is None:
        hit = _prepare_spmd(edge_index, n, npc)
        _CACHE[("prep", pkey)] = hit
    struct, per_core = hit

    ckey = (n, f_in, f_hid, f_out, struct["nchunk_g"], struct["max_nb"],
            tuple(struct["ranges"]))
    if ckey not in _CACHE:
        _CACHE[ckey] = _build(struct, n, npc, f_in, f_hid, f_out)
    nc = _CACHE[ckey]

    ones40 = np.ones((f_out, 1), np.float32)
    in_maps = []
    for c in range(N_CORES):
        pc = per_core[c]
        in_maps.append({
            "xT": np.ascontiguousarray(x[c * npc: (c + 1) * npc].T).astype(np.float16),
            "W1": np.asarray(W1, np.float16), "b1": np.asarray(b1, np.float32).reshape(-1, 1),
            "W2": np.asarray(W2, np.float16), "b2": np.asarray(b2, np.float32).reshape(-1, 1),
            "W3": np.asarray(W3, np.float16),
            # -8 shift: log_softmax is shift-invariant; keeps fp16 exp in range
            "b3": np.asarray(b3, np.float32).reshape(-1, 1) - 8.0,
            "idx_all": pc["idx16"], "S_all": pc["S"], "dis_col": pc["dis_col"],
            "ones40": ones40, "ones40h": ones40.astype(np.float16),
        })
    kw = {}
    if TRACE:
        import tempfile
        kw = dict(trace=True, trace_cores=[0],
                  tmpdir=tempfile.mkdtemp(prefix="gcn_v3_"))
    res = run_bass_kernel_spmd(nc, in_maps, core_ids=list(range(N_CORES)), **kw)
    LAST_RESULT = res
    out = np.concatenate(
        [res.results[c]["out3T"].T for c in range(N_CORES)], axis=0
    ).astype(np.float32)
    return out
